# revision 28
# baseline (speedup 1.0000x reference)
"""Trainium2 Bass kernel for per-batch 2-center cosine k-means (6 iterations).

Strategy: pure data-parallel over the batch dim B=8 -> one batch per NeuronCore,
no collectives.  Each core runs the full 6-iteration loop for its image:

  - D-pass: dists via PE matmuls with X-chunks [128ch x 128pt] as the stationary
    operand and the (normalized, transposed) centers [128, 2] streaming, so the
    distances land points-on-partitions - the layout every downstream
    elementwise op and the mask matmul need.
  - stats/masks: DVE fused ops + two tiny cross-partition PE matmuls.
  - T-pass: centersIter via 512 accumulating PE matmuls with mask-chunks
    [128pt x 2] stationary and pre-transposed X rows [128pt x 129] streaming
    (129th column of ones yields the mask counts for free).

X in the transposed (points-major) layout is prepared host-side; ~19.6 MiB of it
stays SBUF-resident, the rest streams from HBM each iteration together with the
full channels-major copy.

Host post-processing derives onehot/Weight/Cinidist/centersIterout from the
device outputs (final dists, labels at t=1 and t=5, centersIter at t=0 and t=5).
"""

import os

import numpy as np

B = 8
C = 128
N = 65536
NCHUNK = 512          # N / 128 point-chunks
CH1 = C + 1           # channels + ones column
KRES = 288            # SBUF-resident T-layout chunks
TBLK = 8              # streamed T chunks per DMA
DBLK = 16             # D chunks per DMA block (128*2048 tile = 1 MiB)
NUM_ITERS = int(os.environ.get("K_ITERS", "6"))
K_TSTREAM = os.environ.get("K_TSTREAM", "1") == "1"
K_TPASS = os.environ.get("K_TPASS", "1") == "1"
K_STAGE = int(os.environ.get("K_STAGE", "4"))  # 1=D only, 2=+stats, 3=+masks, 4=full
EMA = 0.1

_CACHE = {}


def _build():
    import concourse.mybir as mybir
    import concourse.tile as tile
    from concourse import bacc

    f32 = mybir.dt.float32
    Alu = mybir.AluOpType
    AX = mybir.AxisListType

    nc = bacc.Bacc(None, target_bir_lowering=False)

    f3d = nc.dram_tensor("f3d", (C, N), f32, kind="ExternalInput")
    xt = nc.dram_tensor("xt", (NCHUNK, 128, CH1), f32, kind="ExternalInput")
    rx2_d = nc.dram_tensor("rx2", (128, NCHUNK, 2), f32, kind="ExternalInput")
    cinit_d = nc.dram_tensor("cinit", (2, C), f32, kind="ExternalInput")
    nums_d = nc.dram_tensor("nums", (1, 2), f32, kind="ExternalInput")
    id2_d = nc.dram_tensor("id2", (2, 2), f32, kind="ExternalInput")

    o_dists = nc.dram_tensor("o_dists", (128, NCHUNK, 2), f32, kind="ExternalOutput")
    o_lab = nc.dram_tensor("o_lab", (128, NCHUNK), f32, kind="ExternalOutput")
    o_labp = nc.dram_tensor("o_labp", (128, NCHUNK), f32, kind="ExternalOutput")
    o_ci0 = nc.dram_tensor("o_ci0", (2, C), f32, kind="ExternalOutput")
    o_ci5 = nc.dram_tensor("o_ci5", (2, C), f32, kind="ExternalOutput")

    with tile.TileContext(nc) as tc:
        with (
            tc.tile_pool(name="pers", bufs=1) as pers,
            tc.tile_pool(name="dring", bufs=3) as dring,
            tc.tile_pool(name="tring", bufs=3) as tring,
            tc.tile_pool(name="psd", bufs=1, space="PSUM") as psd,
            tc.tile_pool(name="pss", bufs=1, space="PSUM") as pss,
        ):
            xt_res = pers.tile([128, KRES, CH1], f32, tag="xt_res")
            rx2 = pers.tile([128, NCHUNK, 2], f32, tag="rx2")
            dists = pers.tile([128, NCHUNK, 2], f32, tag="dists")
            masks = pers.tile([128, NCHUNK, 2], f32, tag="masks")
            lab = pers.tile([128, NCHUNK], f32, tag="lab")
            nl = pers.tile([128, NCHUNK], f32, tag="nl")
            dth0 = pers.tile([128, NCHUNK], f32, tag="dth0")
            dth1 = pers.tile([128, NCHUNK], f32, tag="dth1")
            labp = pers.tile([128, NCHUNK], f32, tag="labp")
            stats_p = pers.tile([128, 4], f32, tag="stats_p")
            thr_bc = pers.tile([128, 2], f32, tag="thr_bc")
            ones_col = pers.tile([128, 1], f32, tag="ones_col")
            ones2 = pers.tile([2, 128], f32, tag="ones2")
            id2 = pers.tile([2, 2], f32, tag="id2")
            cent = pers.tile([2, C], f32, tag="cent")
            cn = pers.tile([2, C], f32, tag="cn")
            sq = pers.tile([2, C], f32, tag="sq")
            tmp2 = pers.tile([2, C], f32, tag="tmp2")
            ci = pers.tile([2, C], f32, tag="ci")
            ci0 = pers.tile([2, C], f32, tag="ci0")
            den = pers.tile([2, 1], f32, tag="den")
            nrm = pers.tile([2, 1], f32, tag="nrm")
            s1 = pers.tile([2, 1], f32, tag="s1")
            s2 = pers.tile([2, 1], f32, tag="s2")
            s3 = pers.tile([2, 1], f32, tag="s3")
            sc = pers.tile([1, 8], f32, tag="sc")
            rcp = pers.tile([1, 8], f32, tag="rcp")
            thrs_pad = pers.tile([2, 2], f32, tag="thrs_pad")
            thrs = thrs_pad[0:1, :]
            nums_sb = pers.tile([1, 2], f32, tag="nums_sb")
            rnums = pers.tile([1, 2], f32, tag="rnums")
            rden = pers.tile([2, 1], f32, tag="rden")
            cnT = pers.tile([128, 2], f32, tag="cnT")

            # ---- setup ----
            nc.vector.memset(ones_col[:], 1.0)
            nc.vector.memset(ones2[:], 0.0)
            nc.vector.memset(ones2[0:1, :], 1.0)
            nc.vector.memset(thrs_pad[:], 0.0)
            if K_STAGE < 4:  # bisect builds: keep outputs defined
                nc.vector.memset(lab[:], 0.0)
                nc.vector.memset(labp[:], 0.0)
                nc.vector.memset(ci[:], 0.0)
                nc.vector.memset(ci0[:], 0.0)
                nc.vector.memset(masks[:], 1.0)
                nc.vector.memset(thr_bc[:], 0.0)
            nc.sync.dma_start(rx2[:], rx2_d[:])
            nc.sync.dma_start(cent[:], cinit_d[:])
            nc.sync.dma_start(nums_sb[:], nums_d[:])
            nc.sync.dma_start(id2[:], id2_d[:])
            nc.vector.reciprocal(rnums[:], nums_sb[:])
            if os.environ.get("K_NORES", "0") != "1":
                RESBLK = 38
                for k0 in range(0, KRES, RESBLK):
                    kk = min(RESBLK, KRES - k0)
                    nc.sync.dma_start(
                        xt_res[:, k0 : k0 + kk, :],
                        xt[k0 : k0 + kk, :, :].transpose([1, 0, 2]),
                    )
            else:
                nc.vector.memset(xt_res[:], 0.0)

            for t in range(NUM_ITERS):
                # ---- normalize centers: cn = cent / max(||cent||, 1e-12) ----
                nc.vector.tensor_tensor(sq[:], cent[:], cent[:], op=Alu.mult)
                nc.vector.tensor_reduce(nrm[:], sq[:], axis=AX.X, op=Alu.add)
                nc.scalar.sqrt(s1[:], nrm[:])
                # two Newton steps: s <- 0.5*(s + nrm/s)
                for _ in range(2):
                    nc.vector.reciprocal(s2[:], s1[:])
                    nc.vector.tensor_tensor(s3[:], nrm[:], s2[:], op=Alu.mult)
                    nc.vector.tensor_tensor(s2[:], s3[:], s1[:], op=Alu.add)
                    nc.vector.tensor_scalar(s1[:], s2[:], 0.5, None, op0=Alu.mult)
                nc.vector.tensor_scalar(s3[:], s1[:], 1e-12, None, op0=Alu.max)
                nc.vector.reciprocal(s2[:], s3[:])
                nc.vector.tensor_scalar(cn[:], cent[:], s2[:, 0:1], None, op0=Alu.mult)
                # transpose [2,128] -> [128,2]
                pcnt = pss.tile([128, 2], f32, tag="pcnt")
                nc.tensor.transpose(pcnt[:], cn[:], id2[:])
                nc.vector.tensor_copy(cnT[:], pcnt[:])

                # ---- D-pass: q[p, c, j] = sum_ch X[ch, 128c+p] * cn[j, ch] ----
                pd0 = psd.tile([128, 256, 2], f32, tag="pd0")
                pd1 = psd.tile([128, 256, 2], f32, tag="pd1")
                pd = (pd0, pd1)
                for blk in range(N // (DBLK * 128)):
                    dt_tile = dring.tile([128, DBLK * 128], f32, tag="dblk")
                    nc.sync.dma_start(
                        dt_tile[:], f3d[:, blk * DBLK * 128 : (blk + 1) * DBLK * 128]
                    )
                    for cc in range(DBLK):
                        c = blk * DBLK + cc
                        bank, col = divmod(c, 256)
                        nc.tensor.matmul(
                            pd[bank][:, col, :],
                            dt_tile[:, cc * 128 : (cc + 1) * 128],
                            cnT[:],
                            start=True,
                            stop=True,
                        )
                # evict + scale by 1/||x||: dists = q * rx
                for bank in range(2):
                    nc.vector.tensor_tensor(
                        dists[:, bank * 256 : (bank + 1) * 256, :],
                        pd[bank][:],
                        rx2[:, bank * 256 : (bank + 1) * 256, :],
                        op=Alu.mult,
                    )
                # dists = 0.5 - 0.5*cos
                nc.vector.tensor_scalar(
                    dists[:], dists[:], -0.5, 0.5, op0=Alu.mult, op1=Alu.add
                )

                d0 = dists[:, :, 0]
                d1 = dists[:, :, 1]
                if K_STAGE < 2:
                    continue
                # labels / stats
                nc.vector.tensor_tensor(lab[:], d1, d0, op=Alu.is_lt)
                nc.vector.tensor_tensor(nl[:], d1, d0, op=Alu.is_ge)
                if t == min(1, NUM_ITERS - 1):
                    nc.vector.tensor_copy(labp[:], lab[:])
                nc.vector.tensor_tensor(dth0[:], d0, nl[:], op=Alu.mult)
                nc.vector.tensor_tensor(dth1[:], d1, lab[:], op=Alu.mult)
                nc.vector.tensor_reduce(
                    stats_p[:, 0:1], dth0[:], axis=AX.X, op=Alu.add
                )
                nc.vector.tensor_reduce(
                    stats_p[:, 1:2], dth1[:], axis=AX.X, op=Alu.add
                )
                nc.vector.tensor_reduce(
                    stats_p[:, 2:3], lab[:], axis=AX.X, op=Alu.add
                )
                # cross-partition totals: [1,3] = ones^T @ stats
                pst = pss.tile([1, 4], f32, tag="pst")
                nc.tensor.matmul(
                    pst[:, 0:3], ones_col[:], stats_p[:, 0:3], start=True, stop=True
                )
                if K_STAGE < 3:
                    continue
                # scalar chain (partition 0): thresholds
                nc.vector.tensor_scalar(sc[:, 0:1], pst[:, 2:3], 1.0, None, op0=Alu.add)
                nc.vector.tensor_scalar(
                    sc[:, 1:2], pst[:, 2:3], -1.0, float(N + 1), op0=Alu.mult, op1=Alu.add
                )
                nc.vector.reciprocal(rcp[:, 0:2], sc[:, 0:2])
                nc.vector.tensor_tensor(sc[:, 2:3], pst[:, 1:2], rcp[:, 0:1], op=Alu.mult)
                nc.vector.tensor_tensor(sc[:, 3:4], pst[:, 0:1], rcp[:, 1:2], op=Alu.mult)
                nc.vector.tensor_tensor(thrs[:, 1:2], sc[:, 2:3], rnums[:, 0:1], op=Alu.mult)
                nc.vector.tensor_tensor(thrs[:, 0:1], sc[:, 3:4], nums_sb[:, 1:2], op=Alu.mult)
                # broadcast thresholds to all partitions (K=2; row 1 is zeros)
                pthr = pss.tile([128, 2], f32, tag="pthr")
                nc.tensor.matmul(pthr[:], ones2[:], thrs_pad[:], start=True, stop=True)
                nc.vector.tensor_copy(thr_bc[:], pthr[:])
                # masks: m = (dth <= thr) * indicator
                nc.vector.tensor_scalar(
                    dth0[:], dth0[:], thr_bc[:, 0:1], None, op0=Alu.is_le
                )
                nc.vector.tensor_scalar(
                    dth1[:], dth1[:], thr_bc[:, 1:2], None, op0=Alu.is_le
                )
                nc.vector.tensor_tensor(masks[:, :, 0], dth0[:], nl[:], op=Alu.mult)
                nc.vector.tensor_tensor(masks[:, :, 1], dth1[:], lab[:], op=Alu.mult)

                if K_STAGE < 4:
                    continue
                # ---- T-pass: centersIter sums + counts ----
                pci = pss.tile([2, CH1], f32, tag="pci")
                tt = None
                nchunk_t = NCHUNK if (K_TSTREAM and K_TPASS) else KRES
                for c in range(nchunk_t):
                    if c < KRES:
                        rhs = xt_res[:, c, :]
                    else:
                        k = c - KRES
                        if k % TBLK == 0:
                            tt = tring.tile([128, TBLK, CH1], f32, tag="tblk")
                            nc.sync.dma_start(
                                tt[:], xt[c : c + TBLK, :, :].transpose([1, 0, 2])
                            )
                        rhs = tt[:, k % TBLK, :]
                    nc.tensor.matmul(
                        pci[:],
                        masks[:, c, :] if K_TPASS else thr_bc[:, 0:2],
                        rhs,
                        start=(c == 0),
                        stop=(c == nchunk_t - 1),
                    )
                # centersIter = sums / (counts + 1)
                nc.vector.tensor_scalar(den[:], pci[:, C : C + 1], 1.0, None, op0=Alu.add)
                nc.vector.reciprocal(rden[:], den[:])
                nc.vector.tensor_scalar(ci[:], pci[:, 0:C], rden[:, 0:1], None, op0=Alu.mult)
                if t == 0:
                    nc.vector.tensor_copy(ci0[:], ci[:])
                if t < NUM_ITERS - 1:
                    # EMA: cent = cent + 0.1*(ci - cent)
                    nc.vector.tensor_tensor(tmp2[:], ci[:], cent[:], op=Alu.subtract)
                    nc.vector.tensor_scalar(tmp2[:], tmp2[:], EMA, None, op0=Alu.mult)
                    nc.vector.tensor_tensor(cent[:], tmp2[:], cent[:], op=Alu.add)

            # ---- outputs ----
            nc.sync.dma_start(o_dists[:], dists[:])
            nc.sync.dma_start(o_lab[:], lab[:])
            nc.sync.dma_start(o_labp[:], labp[:])
            nc.sync.dma_start(o_ci0[:], ci0[:])
            nc.sync.dma_start(o_ci5[:], ci[:])

    nc.compile()
    return nc


def _get_nc():
    if "nc" not in _CACHE:
        _CACHE["nc"] = _build()
    return _CACHE["nc"]


def kernel(FeatureT, centerInit, num1, num2):
    from concourse.bass_utils import run_bass_kernel_spmd

    F3 = np.ascontiguousarray(np.asarray(FeatureT, dtype=np.float32)).reshape(B, C, N)
    cinit = np.ascontiguousarray(np.asarray(centerInit, dtype=np.float32))

    # T-layout with ones column: [B, NCHUNK, 128, 129]
    XT = np.empty((B, NCHUNK, 128, CH1), dtype=np.float32)
    XT[..., :C] = F3.transpose(0, 2, 1).reshape(B, NCHUNK, 128, C)
    XT[..., C] = 1.0

    # reciprocal row norms (f64 -> f32), pts-on-partitions layout, repeated x2
    ssq = (F3.astype(np.float64) ** 2).sum(axis=1)                  # [B, N]
    rx = (1.0 / np.maximum(np.sqrt(ssq), 1e-12)).astype(np.float32)  # [B, N]
    rx2 = np.repeat(
        rx.reshape(B, NCHUNK, 128).transpose(0, 2, 1)[..., None], 2, axis=-1
    )  # [B, 128, NCHUNK, 2]

    nums = np.array([[float(num1), float(num2)]], dtype=np.float32)
    id2 = np.eye(2, dtype=np.float32)

    in_maps = [
        {
            "f3d": np.ascontiguousarray(F3[b]),
            "xt": np.ascontiguousarray(XT[b]),
            "rx2": np.ascontiguousarray(rx2[b]),
            "cinit": cinit,
            "nums": nums,
            "id2": id2,
        }
        for b in range(B)
    ]

    nc = _get_nc()
    try:
        kr = run_bass_kernel_spmd(nc, in_maps, core_ids=list(range(B)))
    except ModuleNotFoundError:
        # NTFF trace hook unavailable in this environment; run untraced.
        os.environ["BASS_NEVER_TRACE"] = "1"
        kr = run_bass_kernel_spmd(nc, in_maps, core_ids=list(range(B)))
    _CACHE["last_results"] = kr
    res = kr.results

    # ---- host-side unshard + derived outputs ----
    # device layout [128, NCHUNK, x] with point = 128*c + p -> [c, p, x]
    d = np.stack([r["o_dists"] for r in res]).transpose(0, 2, 1, 3).reshape(B, N, 2)
    labf = np.stack([r["o_lab"] for r in res]).transpose(0, 2, 1).reshape(B, N)
    labpf = np.stack([r["o_labp"] for r in res]).transpose(0, 2, 1).reshape(B, N)
    ci5 = np.stack([r["o_ci5"] for r in res])  # [B, 2, C]
    ci0_last = res[-1]["o_ci0"]                # [2, C] (last batch)

    labels = labf.astype(np.int32)
    labelPinit = labpf.astype(np.int32)

    onehot = np.empty((B, N, 2), dtype=np.float32)
    onehot[..., 0] = np.float32(1.0) - labf
    onehot[..., 1] = labf

    dmin = d.min(axis=1, keepdims=True)
    dmax = d.max(axis=1, keepdims=True)
    Weight = np.float32(1.0) - (d - dmin) / (dmax - dmin + np.float32(1e-7))

    centersIterout = ci5.sum(axis=0) / np.float32(B)

    a = ci0_last.astype(np.float32)
    num = (a * cinit).sum(axis=1)
    dn = np.maximum(
        np.linalg.norm(a, axis=1) * np.linalg.norm(cinit, axis=1), np.float32(1e-8)
    )
    Cinidist = np.float32((num / dn).sum() / np.float32(B))

    return (
        centersIterout.astype(np.float32),
        labels,
        onehot,
        Weight.astype(np.float32),
        d,
        labelPinit,
        Cinidist,
    )
